# revision 71
# baseline (speedup 1.0000x reference)
"""CRF-as-RNN mean-field kernel for Trainium2 (Bass/Tile), 8-core SPMD.

Strategy:
- Shard 2 images x 4 row-strips across 8 cores. Each core gets 84 rows
  (64 owned + halo); 5 mean-field iterations shrink the valid region by
  2 rows/iter, so no inter-core communication is needed at all.
- On-chip layout: partitions = 6 row-groups x 21 channels = 126; free dim
  = 14 rows x 256 cols (+2-row/-col halos for in-tile shifted reads:
  18 row-slots x 260 col-slots). Image-boundary zero padding is realized
  by statically-zero halo slots; intra-core group halos are refreshed
  once per iteration with two SBUF->SBUF DMAs.
- The 5x5 spatial gaussian (sigma=0.1) is a numerical delta in f32, so
  sp == q; it is folded into the center-tap constant.
- Bilateral 24-tap MAC runs on DVE in fp16 (2x mode), 12 unique weight
  maps (opposite taps share maps by symmetry). The last tap + center
  fold run column-chunk-wise and the dy=0 taps split at the row-8
  boundary, so softmax (PE/ACT) pipelines under bilateral (DVE) work.
- Softmax runs in 1024-wide chunk groups through PSUM, all-f16 matmuls
  (f16 logits copy, f16 identity/bneg): z = logits - compat-transform;
  exp/ln on ACT; normalization via the exp(z - lnD) trick. Chunk 6
  first so halo-refresh DMAs run under the remaining groups. q0 runs
  before the w-precompute to fill the DMA warm-up window.
- Bilateral color weights precompute: host-f16 img replicated to
  partition bases 0/32/64 (3 parallel DGE queues); diffs on DVE,
  squares split DVE/ACT; one [82,72]-stationary f16 matmul reduces 3
  taps at once into a [72,2340] 5-bank PSUM stack (accumulate-over-
  zeros), so one ACT exp(-50*d2) covers 12 taps; 21-channel broadcast
  via per-tap f16 mask matmuls (spatial weight folded into the mask);
  PSUM->SBUF copies alternate ACT/DVE.
"""

import math
import sys
from contextlib import ExitStack

import numpy as np

sys.path.insert(0, "/opt/trn_rl_repo")

# ---------------- problem constants (hardcoded per contract) ----------------
B, C, H, W = 2, 21, 256, 256
G, RG = 6, 14                  # row groups per strip, rows per group
P = G * C                      # 126 partitions
F = RG * W                     # 3584 free elems (real pixels per partition)
NT, NV = 18, 260               # q/w tile row slots (-2..15), col slots (-2..257)
IU, IV = 22, 264               # img tile row slots (-4..17), col slots (-4..259)
STARTS = [0, 54, 118, 172]     # strip start rows
OWN = [(0, 64), (10, 74), (10, 74), (20, 84)]  # owned local-row range per strip
NUM_ITERS = 5
NCH, CH = 7, 512               # softmax chunks (512 px = 2 rows)
NPC, CP = 10, 468              # w-precompute chunks over NT*NV=4680

# spatial gaussian (5x5, sigma=5), normalized
_ax = np.arange(5, dtype=np.float64) - 2
_xx, _yy = np.meshgrid(_ax, _ax, indexing="ij")
_g = np.exp(-(_xx**2 + _yy**2) / (2 * 5.0**2))
SW = (_g / _g.sum()).astype(np.float64)
WC = float(SW[2, 2])           # center weight (spatial only; color=1 at center)
# 12 unique taps (positive half-window); opposite taps share weight maps
TAPS = [(0, 1), (0, 2), (1, -2), (1, -1), (1, 0), (1, 1), (1, 2),
        (2, -2), (2, -1), (2, 0), (2, 1), (2, 2)]

_BASS_CACHE = {}


def _build_bass():
    import concourse.bass as bass
    import concourse.mybir as mybir
    from concourse import tile

    f32 = mybir.dt.float32
    f16 = mybir.dt.float16
    AF = mybir.ActivationFunctionType
    OP = mybir.AluOpType

    nc = bass.Bass("TRN2", target_bir_lowering=False, debug=False,
                   enable_asserts=False)

    lg_d = nc.dram_tensor("lg", [P, F], f32, kind="ExternalInput")
    img_d = nc.dram_tensor("img", [18, IU * IV], f16, kind="ExternalInput")
    mneg_d = nc.dram_tensor("mneg", [P, P], f16, kind="ExternalInput")
    mneg2_d = nc.dram_tensor("mneg2", [P, P], f16, kind="ExternalInput")
    iden_d = nc.dram_tensor("iden", [P, P], f16, kind="ExternalInput")
    onesd_d = nc.dram_tensor("onesd", [P, G], f16, kind="ExternalInput")
    chms_d = nc.dram_tensor("chms", [82, 72 * 4], f16, kind="ExternalInput")
    bneg_d = nc.dram_tensor("bneg", [G, P], f16, kind="ExternalInput")
    bpos_d = nc.dram_tensor("bpos", [72, P * 12], f16, kind="ExternalInput")
    qout_d = nc.dram_tensor("qout", [P, F], f32, kind="ExternalOutput")

    with tile.TileContext(nc) as tc, ExitStack() as ctx:
        const_pool = ctx.enter_context(tc.tile_pool(name="const", bufs=1))
        main_pool = ctx.enter_context(tc.tile_pool(name="main", bufs=1))
        w_pool = ctx.enter_context(tc.tile_pool(name="wmaps", bufs=1))

        mneg_t = const_pool.tile([P, P], f16, tag="mneg")
        nc.sync.dma_start(mneg_t[:], mneg_d.ap())
        mneg2_t = const_pool.tile([P, P], f16, tag="mneg2")
        nc.sync.dma_start(mneg2_t[:], mneg2_d.ap())
        iden_t = const_pool.tile([P, P], f16, tag="iden")
        nc.sync.dma_start(iden_t[:], iden_d.ap())
        onesd_t = const_pool.tile([P, G], f16, tag="onesd")
        nc.sync.dma_start(onesd_t[:], onesd_d.ap())
        chms_t = const_pool.tile([82, 72 * 4], f16, tag="chms")
        nc.sync.dma_start(chms_t[:], chms_d.ap())
        bneg_t = const_pool.tile([G, P], f16, tag="bneg")
        nc.sync.dma_start(bneg_t[:], bneg_d.ap())
        bpos_t = const_pool.tile([72, P * 12], f16, tag="bpos")
        nc.sync.dma_start(bpos_t[:], bpos_d.ap())

        # Absorber matmuls: each PE matmul can carry only ~1 sync wait
        # beyond its own-engine wait, so pre-observe every stationary's DMA
        # queue with a 2-column dummy matmul (self-referential rhs => the
        # dummy itself waits on exactly one DMA sem).
        with tc.tile_pool(name="scrp", bufs=1, space="PSUM") as scrp:
            scr = scrp.tile([G, 2], f32, tag="scr")
            nc.tensor.matmul(scr[:1, :], mneg_t[:, 0:1], mneg_t[:, 0:2],
                             start=True, stop=True)
            nc.tensor.matmul(scr[:1, :], mneg2_t[:, 0:1], mneg2_t[:, 0:2],
                             start=True, stop=True)
            nc.tensor.matmul(scr[:1, :], iden_t[:, 0:1], iden_t[:, 0:2],
                             start=True, stop=True)
            nc.tensor.matmul(scr[:, :], onesd_t[:], onesd_t[:, 0:2],
                             start=True, stop=True)
            nc.tensor.matmul(scr[:1, :], chms_t[:, 0:1], chms_t[:, 0:2],
                             start=True, stop=True)
            nc.tensor.matmul(scr[:1, :], bneg_t[:, 0:1], bneg_t[:, 0:2],
                             start=True, stop=True)
            nc.tensor.matmul(scr[:1, :], bpos_t[:, 0:1], bpos_t[:, 0:2],
                             start=True, stop=True)

        q_t = main_pool.tile([P, NT * NV], f16, tag="q")
        nc.gpsimd.memset(q_t[:], 0.0)
        q3 = q_t[:].rearrange("p (t v) -> p t v", v=NV)

        w_tiles = [w_pool.tile([P, NT * NV], f16, tag=f"w{i}", name=f"w{i}")
                   for i in range(len(TAPS))]

        lgh_t = main_pool.tile([P, F], f16, tag="lgh")
        e_pool = ctx.enter_context(tc.tile_pool(name="E", bufs=2))
        ln_pool = ctx.enter_context(tc.tile_pool(name="ln", bufs=2))

        # chunk 6 first (it gates the halo-low DMA), then 0 (halo-high):
        # both halo refreshes run under the remaining softmax chunks.
        # Adjacent chunks pair into 1024-wide groups (2-bank PSUM tiles):
        # one exp/ln covers the pair, cutting ACT ops per pass ~25%.
        CORDER = [6, 0, 1, 2, 3, 4, 5]
        CGROUPS = [(6,), (0, 1), (2, 3), (4, 5)]

        def softmax_pass(with_s: bool, last: bool, zpool, dpool, ltt=None):
            for grp in CGROUPS:
                gw = CH * len(grp)
                g0 = grp[0] * CH
                z_ps = zpool.tile([P, 2 * CH], f32, tag="z")
                d_ps = dpool.tile([G, 2 * CH], f32, tag="D")
                for j, c in enumerate(grp):
                    sl = slice(c * CH, (c + 1) * CH)
                    zo = slice(j * CH, (j + 1) * CH)
                    if with_s:
                        # z = mneg*(taps 0..10) + mneg*(last-tap products)
                        #   + (1+wc)*mneg*q (center fold) + logits — the
                        # last-tap adds and center fold ride PE (~20% busy)
                        # instead of the saturated DVE.
                        nc.tensor.matmul(z_ps[:, zo], mneg_t[:],
                                         acc_t[:, sl],
                                         start=True, stop=False,
                                         skip_group_check=True)
                        nc.tensor.matmul(z_ps[:, zo], mneg2_t[:],
                                         q3[:, 2 + 2 * c:4 + 2 * c,
                                            2:2 + W],
                                         start=False, stop=False,
                                         skip_group_check=True)
                        nc.tensor.matmul(z_ps[:, zo], iden_t[:],
                                         lgh_t[:, sl],
                                         start=False, stop=False,
                                         skip_group_check=True)
                    else:
                        nc.tensor.matmul(z_ps[:, zo], iden_t[:],
                                         lgh_t[:, sl],
                                         start=True, stop=False,
                                         skip_group_check=True)
                e_t = e_pool.tile([P, 2 * CH], f16, tag="E")
                nc.scalar.activation(e_t[:, 0:gw], z_ps[:, 0:gw], AF.Exp)
                for j in range(len(grp)):
                    zo = slice(j * CH, (j + 1) * CH)
                    nc.tensor.matmul(d_ps[:, zo], onesd_t[:], e_t[:, zo],
                                     start=True, stop=True,
                                     skip_group_check=True)
                ln_t = ln_pool.tile([G, 2 * CH], f16, tag="ln")
                nc.scalar.activation(ln_t[:, 0:gw], d_ps[:, 0:gw], AF.Ln)
                for j in range(len(grp)):
                    zo = slice(j * CH, (j + 1) * CH)
                    nc.tensor.matmul(z_ps[:, zo], bneg_t[:], ln_t[:, zo],
                                     start=False, stop=True,
                                     skip_group_check=True)
                z3 = z_ps[:].rearrange("p (r x) -> p r x", x=W)
                nr = 2 * len(grp)
                if last:
                    out3 = out_t[:].rearrange("p (r x) -> p r x", x=W)
                    nc.scalar.activation(
                        out3[:, 2 * grp[0]:2 * grp[0] + nr, 0:W],
                        z3[:, 0:nr, 0:W], AF.Exp)
                    nc.sync.dma_start(qout_d.ap()[:, g0:g0 + gw],
                                      out_t[:, g0:g0 + gw])
                else:
                    nc.scalar.activation(
                        q3[:, 2 + 2 * grp[0]:2 + 2 * grp[0] + nr, 2:2 + W],
                        z3[:, 0:nr, 0:W], AF.Exp)

        # ---------------- w-map precompute ----------------
        # Column halves of the (18 x 260) t/v grid: v-rows 0-8 and 9-17
        # (2340 flat cols each). All 12 taps' color distances stack into one
        # [72, 2340] PSUM tile (5 banks; matmul chunks 4x512+292) via output
        # partition offsets with a single shared [18,6] chmask stationary,
        # so ONE exp covers 12 taps. All matmuls f16 (fp32 = 4x slower).
        HR = 9                    # v-rows per half
        HC = HR * NV              # 2340 flat cols per half
        CHK = [(0, 512), (512, 512), (1024, 512), (1536, 512), (2048, 292)]
        with tc.tile_pool(name="pre", bufs=1) as prep, \
             tc.tile_pool(name="pre2", bufs=3) as prep2:
            # f16 img (host-converted) replicated to partition bases 0/32/64
            # (engine accesses must be 32-aligned) so one [82,72]-stationary
            # reduce matmul contracts 3 taps at once (4 mm-groups/chunk
            # instead of 12). Gap partitions 18-31/50-63 of the sq tiles are
            # zeroed once so the matmul contracts zeros there.
            imgh_t = prep.tile([82, IU * IV], f16, tag="imgh")
            # three independent DRAM loads, issued FIRST on their engine
            # streams (before q0's ACT chain) so they start at t~0
            nc.scalar.dma_start(imgh_t[0:18, :], img_d.ap())
            nc.gpsimd.dma_start(imgh_t[32:50, :], img_d.ap())
            nc.scalar.dma_start(imgh_t[64:82, :], img_d.ap())
            img3 = imgh_t[:].rearrange("p (u v) -> p u v", v=IV)
            e_all = prep.tile([72, NT * NV], f16, tag="eall")

            # q0 = softmax(logits) runs BEFORE the w-map compute: it fills
            # the otherwise-idle PE/ACT while the img DMAs + diffs warm up.
            with tc.tile_pool(name="lgld", bufs=1) as lgld:
                lg_t = lgld.tile([P, F], f32, tag="lg")
                nc.sync.dma_start(lg_t[:], lg_d.ap())
                nc.scalar.copy(lgh_t[:], lg_t[:])  # f16 for fast PE reads
            with tc.tile_pool(name="zq", bufs=3, space="PSUM") as zq_pool, \
                 tc.tile_pool(name="dq", bufs=1, space="PSUM") as dq_pool:
                softmax_pass(with_s=False, last=False,
                             zpool=zq_pool, dpool=dq_pool)

            diff_b = [prep.tile([82, HC], f16, tag=f"diff{i}",
                                name=f"diff{i}") for i in range(3)]
            sq_b = [prep.tile([82, HC], f16, tag=f"sq{i}", name=f"sq{i}")
                    for i in range(3)]
            for sq_t in sq_b:
                nc.gpsimd.memset(sq_t[:], 0.0)

            pctx = ExitStack()
            psd = pctx.enter_context(tc.tile_pool(name="psd", bufs=1,
                                                  space="PSUM"))
            psw = pctx.enter_context(tc.tile_pool(name="psw", bufs=3,
                                                  space="PSUM"))
            # phase A: diffs/squares/reduces + exps for BOTH halves, so DVE
            # runs continuously (no copy ops blocking half-1 diffs in its
            # program order); phase B: all broadcasts + PSUM->SBUF copies.
            for half in range(2):
                r0 = half * HR
                c0 = half * HC
                # padded to 5 full PSUM banks so psw tiles stay bank-aligned
                d2h = psd.tile([72, 2560], f32, tag="d2")
                for tg in range(4):          # tap groups of 3
                    diff_t = diff_b[(half * 4 + tg) % 3]
                    sq_t = sq_b[(half * 4 + tg) % 3]
                    diff3 = diff_t[:].rearrange("p (t v) -> p t v", v=NV)
                    for tt in range(3):
                        dy, dx = TAPS[3 * tg + tt]
                        po = 32 * tt
                        nc.vector.tensor_sub(
                            diff3[po:po + 18, 0:HR, 0:NV],
                            img3[po:po + 18,
                                 2 + r0 + dy:2 + r0 + dy + HR,
                                 2 + dx:2 + dx + NV],
                            img3[po:po + 18, 2 + r0:2 + r0 + HR, 2:2 + NV],
                        )
                        if tt < 2:
                            nc.vector.tensor_mul(sq_t[po:po + 18, :],
                                                 diff_t[po:po + 18, :],
                                                 diff_t[po:po + 18, :])
                        else:
                            nc.scalar.square(sq_t[po:po + 18, :],
                                             diff_t[po:po + 18, :])
                    for o, w_ in CHK:
                        nc.tensor.matmul(d2h[:, o:o + w_],
                                         chms_t[:, 72 * tg:72 * tg + 72],
                                         sq_t[:, o:o + w_],
                                         start=(tg == 0), stop=(tg == 3),
                                         skip_group_check=True)
                nc.scalar.activation(e_all[:, c0:c0 + HC], d2h[:, 0:HC],
                                     AF.Exp, scale=-50.0)
            # tap-outer so each w map completes (both halves) before the
            # next; copies all on ACT so DVE is free to start iteration 1's
            # bilateral products as soon as each w map lands — phase B then
            # hides under iteration-1 DVE work.
            for ki in range(len(TAPS)):
                for half in range(2):
                    c0 = half * HC
                    for o, w_ in CHK:
                        w_ps = psw.tile([P, 512], f32, tag="wps")
                        nc.tensor.matmul(
                            w_ps[:, 0:w_],
                            bpos_t[:, ki * P:(ki + 1) * P],
                            e_all[:, c0 + o:c0 + o + w_],
                            start=True, stop=True)
                        nc.scalar.copy(
                            w_tiles[ki][:, c0 + o:c0 + o + w_],
                            w_ps[:, 0:w_])
            pctx.close()

        zps_pool = ctx.enter_context(tc.tile_pool(name="zps", bufs=3,
                                                  space="PSUM"))
        dps_pool = ctx.enter_context(tc.tile_pool(name="dps", bufs=1,
                                                  space="PSUM"))

        # ---------------- iteration tiles ----------------
        post_pool = ctx.enter_context(tc.tile_pool(name="post", bufs=1))
        acc_t = post_pool.tile([P, F], f16, tag="acc")
        acc3 = acc_t[:].rearrange("p (r x) -> p r x", x=W)
        out_t = post_pool.tile([P, F], f32, tag="out")
        tmp_pool = ctx.enter_context(tc.tile_pool(name="tmp", bufs=2))
        tmpc_pool = ctx.enter_context(tc.tile_pool(name="tmpc", bufs=8))

        for it in range(NUM_ITERS):
            last = it == NUM_ITERS - 1
            # refresh intra-core group halos (2 SBUF->SBUF DMAs)
            nc.sync.dma_start(q3[21:126, 0:2, 0:NV], q3[0:105, 14:16, 0:NV])
            nc.sync.dma_start(q3[0:105, 16:18, 0:NV], q3[21:126, 2:4, 0:NV])

            # bilateral: 24 taps = 12 unique maps x {gather, scatter-sym}.
            # The dy=0 taps (0,1) split at the row-8 boundary: their top
            # halves only need softmax groups (0,1)/(2,3) of the previous
            # pass, so DVE restarts before the last group drains.
            first = True
            for reg, (ra, rb) in (("A", (0, 8)), ("B", (8, 14))):
                for ki in (0, 1):
                    dy, dx = TAPS[ki]
                    w3 = w_tiles[ki][:].rearrange("p (t v) -> p t v", v=NV)
                    nr = rb - ra
                    for (qdy, qdx, wdy, wdx) in ((dy, dx, 0, 0),
                                                 (-dy, -dx, -dy, -dx)):
                        q_ap = q3[:, 2 + qdy + ra:2 + qdy + rb,
                                  2 + qdx:2 + qdx + W]
                        w_ap = w3[:, 2 + wdy + ra:2 + wdy + rb,
                                  2 + wdx:2 + wdx + W]
                        if ki == 0 and qdy == dy and qdx == dx:
                            nc.vector.tensor_mul(acc3[:, ra:rb, 0:W],
                                                 q_ap, w_ap)
                        else:
                            t = tmp_pool.tile([P, F], f16, tag="tmp")
                            t3 = t[:].rearrange("p (r x) -> p r x", x=W)
                            nc.vector.tensor_mul(t3[:, ra:rb, 0:W],
                                                 q_ap, w_ap)
                            nc.vector.tensor_add(
                                acc3[:, ra:rb, 0:W], acc3[:, ra:rb, 0:W],
                                t3[:, ra:rb, 0:W])
            for ki, (dy, dx) in enumerate(TAPS[:-1]):
                if ki in (0, 1):
                    continue
                w3 = w_tiles[ki][:].rearrange("p (t v) -> p t v", v=NV)
                for (qdy, qdx, wdy, wdx) in ((dy, dx, 0, 0),
                                             (-dy, -dx, -dy, -dx)):
                    q_ap = q3[:, 2 + qdy:2 + qdy + RG, 2 + qdx:2 + qdx + W]
                    w_ap = w3[:, 2 + wdy:2 + wdy + RG, 2 + wdx:2 + wdx + W]
                    t = tmp_pool.tile([P, F], f16, tag="tmp")
                    t3 = t[:].rearrange("p (r x) -> p r x", x=W)
                    nc.vector.tensor_mul(t3[:, 0:RG, 0:W], q_ap, w_ap)
                    nc.vector.tensor_add(acc_t[:], acc_t[:], t[:])

            # last tap runs column-chunk-wise, products only (DVE muls into
            # per-chunk tiles); the accumulate-adds and the center fold ride
            # PE matmuls inside the softmax z-group, so softmax chunk c
            # starts while DVE still works on chunk c+1.
            dy, dx = TAPS[-1]
            w3 = w_tiles[-1][:].rearrange("p (t v) -> p t v", v=NV)
            ltt = {}
            for c in CORDER:
                sl = slice(c * CH, (c + 1) * CH)
                rr = 2 * c
                pair = []
                for (qdy, qdx, wdy, wdx) in ((dy, dx, 0, 0),
                                             (-dy, -dx, -dy, -dx)):
                    t = tmpc_pool.tile([P, CH], f16, tag="tmpc")
                    t3 = t[:].rearrange("p (r x) -> p r x", x=W)
                    nc.vector.tensor_mul(
                        t3[:, 0:2, 0:W],
                        q3[:, 2 + qdy + rr:4 + qdy + rr, 2 + qdx:2 + qdx + W],
                        w3[:, 2 + wdy + rr:4 + wdy + rr, 2 + wdx:2 + wdx + W])
                    nc.vector.tensor_add(acc_t[:, sl], acc_t[:, sl], t[:])
                    pair.append(t)
                ltt[c] = pair

            softmax_pass(with_s=True, last=last,
                         zpool=zps_pool, dpool=dps_pool, ltt=ltt)

    _legalize_matmul_waits(nc, mybir)
    return nc


def _legalize_matmul_waits(nc, mybir, max_waits=2):
    """TRN2 ISA sync-wait structs hold few waits per instruction (2 for PE
    matmult/NoOp, 1 for DVE TensorTensor, ...); codegen aborts on more.
    Move excess waits onto InstNoOps (1 wait each) inserted right before
    on the same engine (adjacent => identical blocking semantics)."""
    cap = {}
    for f in nc.m.functions:
        for blk in f.blocks:
            insts = blk.instructions
            out = []
            changed = False
            for i in insts:
                si = getattr(i, "sync_info", None)
                eng = getattr(i, "engine", None)
                max_waits = cap.get(type(i).__name__, 1)
                if (si is not None and eng is not None
                        and len(si.on_wait) > max_waits):
                    waits = list(si.on_wait)
                    keep, move = [], []
                    for w in waits:
                        if "PE" in w.ant_name and len(keep) < max_waits:
                            keep.append(w)
                        else:
                            move.append(w)
                    while len(keep) < max_waits and move:
                        keep.append(move.pop())
                    nop_cap = cap.get("InstNoOp", 1)
                    while move:
                        grp, move = move[:nop_cap], move[nop_cap:]
                        nop = mybir.InstNoOp(
                            name=nc.get_next_instruction_name(),
                            engine=eng, ins=[], outs=[])
                        nop.sync_info = mybir.SyncInfo(on_wait=grp,
                                                       on_update=[])
                        out.append(nop)
                    i.sync_info = mybir.SyncInfo(
                        on_wait=keep, on_update=list(si.on_update))
                    changed = True
                out.append(i)
            if changed:
                blk.instructions = out


def _prep_shards(logits, img, compat):
    """Host-side shard prep -> list of 8 in_maps."""
    mneg = np.kron(np.eye(G), -compat.T.astype(np.float64)).astype(np.float16)
    mneg2 = np.kron(np.eye(G),
                    -(1.0 + WC) * compat.T.astype(np.float64)).astype(np.float16)
    iden = np.eye(P, dtype=np.float16)
    onesd = np.kron(np.eye(G), np.ones((C, 1))).astype(np.float16)
    chms = np.zeros((82, 72 * 4))
    for k in range(12):
        j, t = divmod(k, 3)
        for g in range(G):
            chms[32 * t + 3 * g:32 * t + 3 * g + 3, 72 * j + 6 * k + g] = 1
    chms = chms.astype(np.float16)
    bneg = np.kron(np.eye(G), -np.ones((1, C))).astype(np.float16)
    bpos = np.concatenate(
        [np.concatenate(
            [np.zeros((6 * k, P)),
             np.kron(np.eye(G), float(SW[2 + dy, 2 + dx]) * np.ones((1, C))),
             np.zeros((6 * (11 - k), P))], axis=0)
         for k, (dy, dx) in enumerate(TAPS)], axis=1).astype(np.float16)

    in_maps = []
    for core in range(8):
        b, j = divmod(core, 4)
        s = STARTS[j]
        lg = logits[b, :, s:s + 84, :].reshape(C, G, RG, W)
        lg = np.ascontiguousarray(
            lg.transpose(1, 0, 2, 3).reshape(P, F)).astype(np.float32)
        im = np.zeros((G, 3, IU, IV), np.float32)
        for g in range(G):
            base = s + g * RG - 4
            u0, u1 = max(0, -base), min(IU, H - base)
            im[g, :, u0:u1, 4:4 + W] = img[b, :, base + u0:base + u1, :]
        im = im.reshape(18, IU * IV).astype(np.float16)
        in_maps.append({
            "lg": lg, "img": np.ascontiguousarray(im),
            "mneg": mneg, "mneg2": mneg2, "iden": iden, "onesd": onesd,
            "chms": chms, "bneg": bneg, "bpos": bpos,
        })
    return in_maps


def kernel(**inputs):
    logits = np.asarray(inputs["logits"], dtype=np.float32)
    img = np.asarray(inputs["img"], dtype=np.float32)
    compat = np.asarray(inputs["compat_mat"], dtype=np.float32)

    from concourse.bass_utils import run_bass_kernel_spmd

    if "nc" not in _BASS_CACHE:
        _BASS_CACHE["nc"] = _build_bass()
    nc = _BASS_CACHE["nc"]

    in_maps = _prep_shards(logits, img, compat)
    res = run_bass_kernel_spmd(nc, in_maps, core_ids=list(range(8)))
    _BASS_CACHE["last_result"] = res

    out = np.zeros((B, C, H, W), np.float32)
    for core in range(8):
        b, j = divmod(core, 4)
        s = STARTS[j]
        lo, hi = OWN[j]
        qc = res.results[core]["qout"].reshape(G, C, RG, W)
        qc = qc.transpose(1, 0, 2, 3).reshape(C, 84, W)
        out[b, :, s + lo:s + hi, :] = qc[:, lo:hi, :]
    return out



# revision 72
# speedup vs baseline: 1.2197x; 1.2197x over previous
"""CRF-as-RNN mean-field kernel for Trainium2 (Bass/Tile), 8-core SPMD.

Strategy:
- Shard 2 images x 4 row-strips across 8 cores. Each core gets 84 rows
  (64 owned + halo); 5 mean-field iterations shrink the valid region by
  2 rows/iter, so no inter-core communication is needed at all.
- On-chip layout: partitions = 6 row-groups x 21 channels = 126; free dim
  = 14 rows x 256 cols (+2-row/-col halos for in-tile shifted reads:
  18 row-slots x 260 col-slots). Image-boundary zero padding is realized
  by statically-zero halo slots; intra-core group halos are refreshed
  once per iteration with two SBUF->SBUF DMAs.
- The 5x5 spatial gaussian (sigma=0.1) is a numerical delta in f32, so
  sp == q; it is folded into the center-tap constant.
- Bilateral 24-tap MAC runs on DVE in fp16 (2x mode), 12 unique weight
  maps (opposite taps share maps by symmetry). The last tap + center
  fold run column-chunk-wise and the dy=0 taps split at the row-8
  boundary, so softmax (PE/ACT) pipelines under bilateral (DVE) work.
- Softmax runs in 1024-wide chunk groups through PSUM, all-f16 matmuls
  (f16 logits copy, f16 identity/bneg): z = logits - compat-transform;
  exp/ln on ACT; normalization via the exp(z - lnD) trick. Chunk 6
  first so halo-refresh DMAs run under the remaining groups. q0 runs
  before the w-precompute to fill the DMA warm-up window.
- Bilateral color weights precompute: host-f16 img replicated to
  partition bases 0/32/64 (3 parallel DGE queues); diffs on DVE,
  squares split DVE/ACT; one [82,72]-stationary f16 matmul reduces 3
  taps at once into a [72,2340] 5-bank PSUM stack (accumulate-over-
  zeros), so one ACT exp(-50*d2) covers 12 taps; 21-channel broadcast
  via per-tap f16 mask matmuls (spatial weight folded into the mask);
  PSUM->SBUF copies alternate ACT/DVE.
"""

import math
import sys
from contextlib import ExitStack

import numpy as np

sys.path.insert(0, "/opt/trn_rl_repo")

# ---------------- problem constants (hardcoded per contract) ----------------
B, C, H, W = 2, 21, 256, 256
G, RG = 6, 14                  # row groups per strip, rows per group
P = G * C                      # 126 partitions
F = RG * W                     # 3584 free elems (real pixels per partition)
NT, NV = 18, 260               # q/w tile row slots (-2..15), col slots (-2..257)
IU, IV = 22, 264               # img tile row slots (-4..17), col slots (-4..259)
STARTS = [0, 54, 118, 172]     # strip start rows
OWN = [(0, 64), (10, 74), (10, 74), (20, 84)]  # owned local-row range per strip
NUM_ITERS = 5
NCH, CH = 7, 512               # softmax chunks (512 px = 2 rows)
NPC, CP = 10, 468              # w-precompute chunks over NT*NV=4680

# spatial gaussian (5x5, sigma=5), normalized
_ax = np.arange(5, dtype=np.float64) - 2
_xx, _yy = np.meshgrid(_ax, _ax, indexing="ij")
_g = np.exp(-(_xx**2 + _yy**2) / (2 * 5.0**2))
SW = (_g / _g.sum()).astype(np.float64)
WC = float(SW[2, 2])           # center weight (spatial only; color=1 at center)
# 12 unique taps (positive half-window); opposite taps share weight maps
TAPS = [(0, 1), (0, 2), (1, -2), (1, -1), (1, 0), (1, 1), (1, 2),
        (2, -2), (2, -1), (2, 0), (2, 1), (2, 2)]

_BASS_CACHE = {}


def _build_bass():
    import concourse.bass as bass
    import concourse.mybir as mybir
    from concourse import tile

    f32 = mybir.dt.float32
    f16 = mybir.dt.float16
    AF = mybir.ActivationFunctionType
    OP = mybir.AluOpType

    nc = bass.Bass("TRN2", target_bir_lowering=False, debug=False,
                   enable_asserts=False)

    lg_d = nc.dram_tensor("lg", [P, F], f32, kind="ExternalInput")
    img_d = nc.dram_tensor("img", [18, IU * IV], f16, kind="ExternalInput")
    mneg_d = nc.dram_tensor("mneg", [P, P], f16, kind="ExternalInput")
    mneg2_d = nc.dram_tensor("mneg2", [P, P], f16, kind="ExternalInput")
    iden_d = nc.dram_tensor("iden", [P, P], f16, kind="ExternalInput")
    onesd_d = nc.dram_tensor("onesd", [P, G], f16, kind="ExternalInput")
    chms_d = nc.dram_tensor("chms", [82, 72 * 4], f16, kind="ExternalInput")
    bneg_d = nc.dram_tensor("bneg", [G, P], f16, kind="ExternalInput")
    bpos_d = nc.dram_tensor("bpos", [72, P * 12], f16, kind="ExternalInput")
    qout_d = nc.dram_tensor("qout", [P, F], f32, kind="ExternalOutput")

    with tile.TileContext(nc) as tc, ExitStack() as ctx:
        const_pool = ctx.enter_context(tc.tile_pool(name="const", bufs=1))
        main_pool = ctx.enter_context(tc.tile_pool(name="main", bufs=1))
        w_pool = ctx.enter_context(tc.tile_pool(name="wmaps", bufs=1))

        mneg_t = const_pool.tile([P, P], f16, tag="mneg")
        nc.sync.dma_start(mneg_t[:], mneg_d.ap())
        mneg2_t = const_pool.tile([P, P], f16, tag="mneg2")
        nc.sync.dma_start(mneg2_t[:], mneg2_d.ap())
        iden_t = const_pool.tile([P, P], f16, tag="iden")
        nc.sync.dma_start(iden_t[:], iden_d.ap())
        onesd_t = const_pool.tile([P, G], f16, tag="onesd")
        nc.sync.dma_start(onesd_t[:], onesd_d.ap())
        chms_t = const_pool.tile([82, 72 * 4], f16, tag="chms")
        nc.sync.dma_start(chms_t[:], chms_d.ap())
        bneg_t = const_pool.tile([G, P], f16, tag="bneg")
        nc.sync.dma_start(bneg_t[:], bneg_d.ap())
        bpos_t = const_pool.tile([72, P * 12], f16, tag="bpos")
        nc.sync.dma_start(bpos_t[:], bpos_d.ap())

        # Absorber matmuls: each PE matmul can carry only ~1 sync wait
        # beyond its own-engine wait, so pre-observe every stationary's DMA
        # queue with a 2-column dummy matmul (self-referential rhs => the
        # dummy itself waits on exactly one DMA sem).
        with tc.tile_pool(name="scrp", bufs=1, space="PSUM") as scrp:
            scr = scrp.tile([G, 2], f32, tag="scr")
            nc.tensor.matmul(scr[:1, :], mneg_t[:, 0:1], mneg_t[:, 0:2],
                             start=True, stop=True)
            nc.tensor.matmul(scr[:1, :], mneg2_t[:, 0:1], mneg2_t[:, 0:2],
                             start=True, stop=True)
            nc.tensor.matmul(scr[:1, :], iden_t[:, 0:1], iden_t[:, 0:2],
                             start=True, stop=True)
            nc.tensor.matmul(scr[:, :], onesd_t[:], onesd_t[:, 0:2],
                             start=True, stop=True)
            nc.tensor.matmul(scr[:1, :], chms_t[:, 0:1], chms_t[:, 0:2],
                             start=True, stop=True)
            nc.tensor.matmul(scr[:1, :], bneg_t[:, 0:1], bneg_t[:, 0:2],
                             start=True, stop=True)
            nc.tensor.matmul(scr[:1, :], bpos_t[:, 0:1], bpos_t[:, 0:2],
                             start=True, stop=True)

        q_t = main_pool.tile([P, NT * NV], f16, tag="q")
        nc.gpsimd.memset(q_t[:], 0.0)
        q3 = q_t[:].rearrange("p (t v) -> p t v", v=NV)

        w_tiles = [w_pool.tile([P, NT * NV], f16, tag=f"w{i}", name=f"w{i}")
                   for i in range(len(TAPS))]

        lgh_t = main_pool.tile([P, F], f16, tag="lgh")
        e_pool = ctx.enter_context(tc.tile_pool(name="E", bufs=2))
        ln_pool = ctx.enter_context(tc.tile_pool(name="ln", bufs=2))

        # chunk 6 first (it gates the halo-low DMA), then 0 (halo-high):
        # both halo refreshes run under the remaining softmax chunks.
        # Adjacent chunks pair into 1024-wide groups (2-bank PSUM tiles):
        # one exp/ln covers the pair, cutting ACT ops per pass ~25%.
        CORDER = [6, 0, 1, 2, 3, 4, 5]
        CGROUPS = [(6,), (0, 1), (2, 3), (4, 5)]

        def softmax_pass(with_s: bool, last: bool, zpool, dpool, ltt=None):
            for grp in CGROUPS:
                gw = CH * len(grp)
                g0 = grp[0] * CH
                z_ps = zpool.tile([P, 2 * CH], f32, tag="z")
                d_ps = dpool.tile([G, 2 * CH], f32, tag="D")
                for j, c in enumerate(grp):
                    sl = slice(c * CH, (c + 1) * CH)
                    zo = slice(j * CH, (j + 1) * CH)
                    if with_s:
                        # z = mneg*(taps 0..10) + mneg*(last-tap products)
                        #   + (1+wc)*mneg*q (center fold) + logits — the
                        # last-tap adds and center fold ride PE (~20% busy)
                        # instead of the saturated DVE.
                        nc.tensor.matmul(z_ps[:, zo], mneg_t[:],
                                         acc_t[:, sl],
                                         start=True, stop=False,
                                         skip_group_check=True)
                        for t in ltt[c]:
                            nc.tensor.matmul(z_ps[:, zo], mneg_t[:], t[:],
                                             start=False, stop=False,
                                             skip_group_check=True)
                        nc.tensor.matmul(z_ps[:, zo], mneg2_t[:],
                                         q3[:, 2 + 2 * c:4 + 2 * c,
                                            2:2 + W],
                                         start=False, stop=False,
                                         skip_group_check=True)
                        nc.tensor.matmul(z_ps[:, zo], iden_t[:],
                                         lgh_t[:, sl],
                                         start=False, stop=False,
                                         skip_group_check=True)
                    else:
                        nc.tensor.matmul(z_ps[:, zo], iden_t[:],
                                         lgh_t[:, sl],
                                         start=True, stop=False,
                                         skip_group_check=True)
                e_t = e_pool.tile([P, 2 * CH], f16, tag="E")
                nc.scalar.activation(e_t[:, 0:gw], z_ps[:, 0:gw], AF.Exp)
                for j in range(len(grp)):
                    zo = slice(j * CH, (j + 1) * CH)
                    nc.tensor.matmul(d_ps[:, zo], onesd_t[:], e_t[:, zo],
                                     start=True, stop=True,
                                     skip_group_check=True)
                ln_t = ln_pool.tile([G, 2 * CH], f16, tag="ln")
                nc.scalar.activation(ln_t[:, 0:gw], d_ps[:, 0:gw], AF.Ln)
                for j in range(len(grp)):
                    zo = slice(j * CH, (j + 1) * CH)
                    nc.tensor.matmul(z_ps[:, zo], bneg_t[:], ln_t[:, zo],
                                     start=False, stop=True,
                                     skip_group_check=True)
                z3 = z_ps[:].rearrange("p (r x) -> p r x", x=W)
                nr = 2 * len(grp)
                if last:
                    out3 = out_t[:].rearrange("p (r x) -> p r x", x=W)
                    nc.scalar.activation(
                        out3[:, 2 * grp[0]:2 * grp[0] + nr, 0:W],
                        z3[:, 0:nr, 0:W], AF.Exp)
                    nc.sync.dma_start(qout_d.ap()[:, g0:g0 + gw],
                                      out_t[:, g0:g0 + gw])
                else:
                    nc.scalar.activation(
                        q3[:, 2 + 2 * grp[0]:2 + 2 * grp[0] + nr, 2:2 + W],
                        z3[:, 0:nr, 0:W], AF.Exp)

        # ---------------- w-map precompute ----------------
        # Column halves of the (18 x 260) t/v grid: v-rows 0-8 and 9-17
        # (2340 flat cols each). All 12 taps' color distances stack into one
        # [72, 2340] PSUM tile (5 banks; matmul chunks 4x512+292) via output
        # partition offsets with a single shared [18,6] chmask stationary,
        # so ONE exp covers 12 taps. All matmuls f16 (fp32 = 4x slower).
        HR = 9                    # v-rows per half
        HC = HR * NV              # 2340 flat cols per half
        CHK = [(0, 512), (512, 512), (1024, 512), (1536, 512), (2048, 292)]
        with tc.tile_pool(name="pre", bufs=1) as prep, \
             tc.tile_pool(name="pre2", bufs=3) as prep2:
            # f16 img (host-converted) replicated to partition bases 0/32/64
            # (engine accesses must be 32-aligned) so one [82,72]-stationary
            # reduce matmul contracts 3 taps at once (4 mm-groups/chunk
            # instead of 12). Gap partitions 18-31/50-63 of the sq tiles are
            # zeroed once so the matmul contracts zeros there.
            imgh_t = prep.tile([82, IU * IV], f16, tag="imgh")
            # three independent DRAM loads, issued FIRST on their engine
            # streams (before q0's ACT chain) so they start at t~0
            nc.scalar.dma_start(imgh_t[0:18, :], img_d.ap())
            nc.gpsimd.dma_start(imgh_t[32:50, :], img_d.ap())
            nc.scalar.dma_start(imgh_t[64:82, :], img_d.ap())
            img3 = imgh_t[:].rearrange("p (u v) -> p u v", v=IV)
            e_all = prep.tile([72, NT * NV], f16, tag="eall")

            # q0 = softmax(logits) runs BEFORE the w-map compute: it fills
            # the otherwise-idle PE/ACT while the img DMAs + diffs warm up.
            with tc.tile_pool(name="lgld", bufs=1) as lgld:
                lg_t = lgld.tile([P, F], f32, tag="lg")
                nc.sync.dma_start(lg_t[:], lg_d.ap())
                nc.scalar.copy(lgh_t[:], lg_t[:])  # f16 for fast PE reads
            with tc.tile_pool(name="zq", bufs=3, space="PSUM") as zq_pool, \
                 tc.tile_pool(name="dq", bufs=1, space="PSUM") as dq_pool:
                softmax_pass(with_s=False, last=False,
                             zpool=zq_pool, dpool=dq_pool)

            diff_b = [prep.tile([82, HC], f16, tag=f"diff{i}",
                                name=f"diff{i}") for i in range(3)]
            sq_b = [prep.tile([82, HC], f16, tag=f"sq{i}", name=f"sq{i}")
                    for i in range(3)]
            for sq_t in sq_b:
                nc.gpsimd.memset(sq_t[:], 0.0)

            pctx = ExitStack()
            psd = pctx.enter_context(tc.tile_pool(name="psd", bufs=1,
                                                  space="PSUM"))
            psw = pctx.enter_context(tc.tile_pool(name="psw", bufs=3,
                                                  space="PSUM"))
            # phase A: diffs/squares/reduces + exps for BOTH halves, so DVE
            # runs continuously (no copy ops blocking half-1 diffs in its
            # program order); phase B: all broadcasts + PSUM->SBUF copies.
            for half in range(2):
                r0 = half * HR
                c0 = half * HC
                # padded to 5 full PSUM banks so psw tiles stay bank-aligned
                d2h = psd.tile([72, 2560], f32, tag="d2")
                for tg in range(4):          # tap groups of 3
                    diff_t = diff_b[(half * 4 + tg) % 3]
                    sq_t = sq_b[(half * 4 + tg) % 3]
                    diff3 = diff_t[:].rearrange("p (t v) -> p t v", v=NV)
                    for tt in range(3):
                        dy, dx = TAPS[3 * tg + tt]
                        po = 32 * tt
                        nc.vector.tensor_sub(
                            diff3[po:po + 18, 0:HR, 0:NV],
                            img3[po:po + 18,
                                 2 + r0 + dy:2 + r0 + dy + HR,
                                 2 + dx:2 + dx + NV],
                            img3[po:po + 18, 2 + r0:2 + r0 + HR, 2:2 + NV],
                        )
                        if tt < 2:
                            nc.vector.tensor_mul(sq_t[po:po + 18, :],
                                                 diff_t[po:po + 18, :],
                                                 diff_t[po:po + 18, :])
                        else:
                            nc.scalar.square(sq_t[po:po + 18, :],
                                             diff_t[po:po + 18, :])
                    for o, w_ in CHK:
                        nc.tensor.matmul(d2h[:, o:o + w_],
                                         chms_t[:, 72 * tg:72 * tg + 72],
                                         sq_t[:, o:o + w_],
                                         start=(tg == 0), stop=(tg == 3),
                                         skip_group_check=True)
                nc.scalar.activation(e_all[:, c0:c0 + HC], d2h[:, 0:HC],
                                     AF.Exp, scale=-50.0)
            cpy = 0   # copies mostly on ACT; every 4th on DVE
            for half in range(2):
                c0 = half * HC
                for ki in range(len(TAPS)):
                    for o, w_ in CHK:
                        w_ps = psw.tile([P, 512], f32, tag="wps")
                        nc.tensor.matmul(
                            w_ps[:, 0:w_],
                            bpos_t[:, ki * P:(ki + 1) * P],
                            e_all[:, c0 + o:c0 + o + w_],
                            start=True, stop=True)
                        if cpy % 4 == 0:
                            nc.vector.tensor_copy(
                                w_tiles[ki][:, c0 + o:c0 + o + w_],
                                w_ps[:, 0:w_])
                        else:
                            nc.scalar.copy(
                                w_tiles[ki][:, c0 + o:c0 + o + w_],
                                w_ps[:, 0:w_])
                        cpy += 1
            pctx.close()

        zps_pool = ctx.enter_context(tc.tile_pool(name="zps", bufs=3,
                                                  space="PSUM"))
        dps_pool = ctx.enter_context(tc.tile_pool(name="dps", bufs=1,
                                                  space="PSUM"))

        # ---------------- iteration tiles ----------------
        post_pool = ctx.enter_context(tc.tile_pool(name="post", bufs=1))
        acc_t = post_pool.tile([P, F], f16, tag="acc")
        acc3 = acc_t[:].rearrange("p (r x) -> p r x", x=W)
        out_t = post_pool.tile([P, F], f32, tag="out")
        tmp_pool = ctx.enter_context(tc.tile_pool(name="tmp", bufs=2))
        tmpc_pool = ctx.enter_context(tc.tile_pool(name="tmpc", bufs=8))

        for it in range(NUM_ITERS):
            last = it == NUM_ITERS - 1
            # refresh intra-core group halos (2 SBUF->SBUF DMAs)
            nc.sync.dma_start(q3[21:126, 0:2, 0:NV], q3[0:105, 14:16, 0:NV])
            nc.sync.dma_start(q3[0:105, 16:18, 0:NV], q3[21:126, 2:4, 0:NV])

            # bilateral: 24 taps = 12 unique maps x {gather, scatter-sym}.
            # The dy=0 taps (0,1) split at the row-8 boundary: their top
            # halves only need softmax groups (0,1)/(2,3) of the previous
            # pass, so DVE restarts before the last group drains.
            first = True
            for reg, (ra, rb) in (("A", (0, 8)), ("B", (8, 14))):
                for ki in (0, 1):
                    dy, dx = TAPS[ki]
                    w3 = w_tiles[ki][:].rearrange("p (t v) -> p t v", v=NV)
                    nr = rb - ra
                    for (qdy, qdx, wdy, wdx) in ((dy, dx, 0, 0),
                                                 (-dy, -dx, -dy, -dx)):
                        q_ap = q3[:, 2 + qdy + ra:2 + qdy + rb,
                                  2 + qdx:2 + qdx + W]
                        w_ap = w3[:, 2 + wdy + ra:2 + wdy + rb,
                                  2 + wdx:2 + wdx + W]
                        if ki == 0 and qdy == dy and qdx == dx:
                            nc.vector.tensor_mul(acc3[:, ra:rb, 0:W],
                                                 q_ap, w_ap)
                        else:
                            t = tmp_pool.tile([P, F], f16, tag="tmp")
                            t3 = t[:].rearrange("p (r x) -> p r x", x=W)
                            nc.vector.tensor_mul(t3[:, ra:rb, 0:W],
                                                 q_ap, w_ap)
                            nc.vector.tensor_add(
                                acc3[:, ra:rb, 0:W], acc3[:, ra:rb, 0:W],
                                t3[:, ra:rb, 0:W])
            for ki, (dy, dx) in enumerate(TAPS[:-1]):
                if ki in (0, 1):
                    continue
                w3 = w_tiles[ki][:].rearrange("p (t v) -> p t v", v=NV)
                for (qdy, qdx, wdy, wdx) in ((dy, dx, 0, 0),
                                             (-dy, -dx, -dy, -dx)):
                    q_ap = q3[:, 2 + qdy:2 + qdy + RG, 2 + qdx:2 + qdx + W]
                    w_ap = w3[:, 2 + wdy:2 + wdy + RG, 2 + wdx:2 + wdx + W]
                    t = tmp_pool.tile([P, F], f16, tag="tmp")
                    t3 = t[:].rearrange("p (r x) -> p r x", x=W)
                    nc.vector.tensor_mul(t3[:, 0:RG, 0:W], q_ap, w_ap)
                    nc.vector.tensor_add(acc_t[:], acc_t[:], t[:])

            # last tap runs column-chunk-wise, products only (DVE muls into
            # per-chunk tiles); the accumulate-adds and the center fold ride
            # PE matmuls inside the softmax z-group, so softmax chunk c
            # starts while DVE still works on chunk c+1.
            dy, dx = TAPS[-1]
            w3 = w_tiles[-1][:].rearrange("p (t v) -> p t v", v=NV)
            ltt = {}
            for c in CORDER:
                rr = 2 * c
                pair = []
                for (qdy, qdx, wdy, wdx) in ((dy, dx, 0, 0),
                                             (-dy, -dx, -dy, -dx)):
                    t = tmpc_pool.tile([P, CH], f16, tag="tmpc")
                    t3 = t[:].rearrange("p (r x) -> p r x", x=W)
                    nc.vector.tensor_mul(
                        t3[:, 0:2, 0:W],
                        q3[:, 2 + qdy + rr:4 + qdy + rr, 2 + qdx:2 + qdx + W],
                        w3[:, 2 + wdy + rr:4 + wdy + rr, 2 + wdx:2 + wdx + W])
                    pair.append(t)
                ltt[c] = pair

            softmax_pass(with_s=True, last=last,
                         zpool=zps_pool, dpool=dps_pool, ltt=ltt)

    _legalize_matmul_waits(nc, mybir)
    return nc


def _legalize_matmul_waits(nc, mybir, max_waits=2):
    """TRN2 ISA sync-wait structs hold few waits per instruction (2 for PE
    matmult/NoOp, 1 for DVE TensorTensor, ...); codegen aborts on more.
    Move excess waits onto InstNoOps (1 wait each) inserted right before
    on the same engine (adjacent => identical blocking semantics)."""
    cap = {}
    for f in nc.m.functions:
        for blk in f.blocks:
            insts = blk.instructions
            out = []
            changed = False
            for i in insts:
                si = getattr(i, "sync_info", None)
                eng = getattr(i, "engine", None)
                max_waits = cap.get(type(i).__name__, 1)
                if (si is not None and eng is not None
                        and len(si.on_wait) > max_waits):
                    waits = list(si.on_wait)
                    keep, move = [], []
                    for w in waits:
                        if "PE" in w.ant_name and len(keep) < max_waits:
                            keep.append(w)
                        else:
                            move.append(w)
                    while len(keep) < max_waits and move:
                        keep.append(move.pop())
                    nop_cap = cap.get("InstNoOp", 1)
                    while move:
                        grp, move = move[:nop_cap], move[nop_cap:]
                        nop = mybir.InstNoOp(
                            name=nc.get_next_instruction_name(),
                            engine=eng, ins=[], outs=[])
                        nop.sync_info = mybir.SyncInfo(on_wait=grp,
                                                       on_update=[])
                        out.append(nop)
                    i.sync_info = mybir.SyncInfo(
                        on_wait=keep, on_update=list(si.on_update))
                    changed = True
                out.append(i)
            if changed:
                blk.instructions = out


def _prep_shards(logits, img, compat):
    """Host-side shard prep -> list of 8 in_maps."""
    mneg = np.kron(np.eye(G), -compat.T.astype(np.float64)).astype(np.float16)
    mneg2 = np.kron(np.eye(G),
                    -(1.0 + WC) * compat.T.astype(np.float64)).astype(np.float16)
    iden = np.eye(P, dtype=np.float16)
    onesd = np.kron(np.eye(G), np.ones((C, 1))).astype(np.float16)
    chms = np.zeros((82, 72 * 4))
    for k in range(12):
        j, t = divmod(k, 3)
        for g in range(G):
            chms[32 * t + 3 * g:32 * t + 3 * g + 3, 72 * j + 6 * k + g] = 1
    chms = chms.astype(np.float16)
    bneg = np.kron(np.eye(G), -np.ones((1, C))).astype(np.float16)
    bpos = np.concatenate(
        [np.concatenate(
            [np.zeros((6 * k, P)),
             np.kron(np.eye(G), float(SW[2 + dy, 2 + dx]) * np.ones((1, C))),
             np.zeros((6 * (11 - k), P))], axis=0)
         for k, (dy, dx) in enumerate(TAPS)], axis=1).astype(np.float16)

    in_maps = []
    for core in range(8):
        b, j = divmod(core, 4)
        s = STARTS[j]
        lg = logits[b, :, s:s + 84, :].reshape(C, G, RG, W)
        lg = np.ascontiguousarray(
            lg.transpose(1, 0, 2, 3).reshape(P, F)).astype(np.float32)
        im = np.zeros((G, 3, IU, IV), np.float32)
        for g in range(G):
            base = s + g * RG - 4
            u0, u1 = max(0, -base), min(IU, H - base)
            im[g, :, u0:u1, 4:4 + W] = img[b, :, base + u0:base + u1, :]
        im = im.reshape(18, IU * IV).astype(np.float16)
        in_maps.append({
            "lg": lg, "img": np.ascontiguousarray(im),
            "mneg": mneg, "mneg2": mneg2, "iden": iden, "onesd": onesd,
            "chms": chms, "bneg": bneg, "bpos": bpos,
        })
    return in_maps


def kernel(**inputs):
    logits = np.asarray(inputs["logits"], dtype=np.float32)
    img = np.asarray(inputs["img"], dtype=np.float32)
    compat = np.asarray(inputs["compat_mat"], dtype=np.float32)

    from concourse.bass_utils import run_bass_kernel_spmd

    if "nc" not in _BASS_CACHE:
        _BASS_CACHE["nc"] = _build_bass()
    nc = _BASS_CACHE["nc"]

    in_maps = _prep_shards(logits, img, compat)
    res = run_bass_kernel_spmd(nc, in_maps, core_ids=list(range(8)))
    _BASS_CACHE["last_result"] = res

    out = np.zeros((B, C, H, W), np.float32)
    for core in range(8):
        b, j = divmod(core, 4)
        s = STARTS[j]
        lo, hi = OWN[j]
        qc = res.results[core]["qout"].reshape(G, C, RG, W)
        qc = qc.transpose(1, 0, 2, 3).reshape(C, 84, W)
        out[b, :, s + lo:s + hi, :] = qc[:, lo:hi, :]
    return out

